# revision 1
# baseline (speedup 1.0000x reference)
"""Causal multi-head attention (B=4, T=2048, C=1024, H=16) on 8 TRN2 cores.

Sharding: batch (4) x head-group (2 groups of 8 heads) -> 8 shards, one per
core. Each core computes QKV projections for its 8 heads, causal flash-style
attention, and a Megatron row-parallel slice of the output projection; the
host sums the two head-group partial outputs per batch element.

Per-core dataflow (all matmuls in float32r, 1 PE cycle/row at N>=256):
  phase 1a: V   = xT c-tiles (lhsT) @ wvT -> [t,dv] -> resident V_aug tiles
  phase 1b: Q^T,K^T = wq/wkT (lhsT) @ xT  -> [f,t]; Q resident, K spilled
  phase 2:  per (head, 512-query block): S^T = K^T.T @ Q^T per 128-k tile
            (diagonal staircase blocks trimmed + packed into one 3-bank psum
            tile), P^T = exp(S^T/8) (ACT; mask multiply on DVE), PV^T
            accumulated with V_aug stationary -> [d+1, q] (row 64 = l),
            normalize via DVE recip + PE ones-broadcast -> ctx^T [c,t]
  phase 3:  y^T = woT (lhsT) @ ctx^T + bias -> [o,t] -> DRAM

Self-contained: hardcodes shapes from the problem spec; no file reads.
"""
import sys
sys.path.insert(0, '/opt/trn_rl_repo')
import numpy as np

B, T, C = 4, 2048, 1024
H, D = 16, 64
N_CORES = 8
HPC = 8        # heads per core
HP = 4         # head pairs per core
KB = 16        # 128-row key tiles per sequence
NQSB = 4       # 512-column query superblocks
CI = 8         # 128-row contraction tiles over C
VW = 66        # V_aug stride per head (64 V + 1 ones + 1 pad)

# Diagonal-staircase packing inside one [128, 1536] psum tile: block j covers
# query range [QOFF[j], 512) of the superblock, lives at psum column POFF[j].
QOFF = (0, 128, 256, 256)
POFF = (0, 512, 896, 1152)
MW = 1408      # merged mask width (gapless staircase packing)

_CACHE = {}


def build_nc(iters=1):
    import contextlib
    import concourse.tile as tile
    from concourse import bacc, mybir

    F32 = mybir.dt.float32
    F32R = mybir.dt.float32r
    EXP = mybir.ActivationFunctionType.Exp
    IDENT = mybir.ActivationFunctionType.Identity

    nc = bacc.Bacc("TRN2", target_bir_lowering=False, debug=False)

    xT_d = nc.dram_tensor("xT", [C, T], F32R, kind="ExternalInput")
    wqT_d = nc.dram_tensor("wqT", [C, 512], F32R, kind="ExternalInput")
    wkT_d = nc.dram_tensor("wkT", [C, 512], F32R, kind="ExternalInput")
    wvT_d = nc.dram_tensor("wvT", [C, 512], F32R, kind="ExternalInput")
    woT_d = nc.dram_tensor("woT", [512, C], F32R, kind="ExternalInput")
    bias_d = nc.dram_tensor("bias", [128, 8], F32, kind="ExternalInput")
    mask_d = nc.dram_tensor("masks", [128, MW], F32R, kind="ExternalInput")
    yT_d = nc.dram_tensor("yT", [C, T], F32, kind="ExternalOutput")
    kT_spill = nc.dram_tensor("kT_spill", [512, T], F32R)

    with tile.TileContext(nc) as tc:
        def emit():
            with contextlib.ExitStack() as es:
                const = es.enter_context(tc.tile_pool(name="const", bufs=1))
                qtp = es.enter_context(tc.tile_pool(name="qt", bufs=1))
                ctxp = es.enter_context(tc.tile_pool(name="ctx", bufs=1))
                vp = es.enter_context(tc.tile_pool(name="vsb", bufs=1))

                ones_f = const.tile([128, 64], F32)
                nc.any.memset(ones_f[:], 1.0)
                ones_r = const.tile([128, 64], F32R)
                nc.vector.tensor_copy(ones_r[:], ones_f[:])
                ones16_f = const.tile([128, 16], F32)
                nc.any.memset(ones16_f[:], 1.0)
                ones16_r = const.tile([128, 16], F32R)
                nc.vector.tensor_copy(ones16_r[:], ones16_f[:])
                bias_sb = const.tile([128, 8], F32)
                nc.sync.dma_start(bias_sb[:], bias_d.ap())

                qt_sb, ctx_sb, v_sb = [], [], []
                for hp in range(HP):
                    qt_sb.append(qtp.tile([128, T], F32R, tag=f"qt{hp}",
                                          name=f"qt{hp}"))
                    ctx_sb.append(ctxp.tile([128, T], F32R, tag=f"ctx{hp}",
                                            name=f"ctx{hp}"))
                for kb in range(KB):
                    v_sb.append(vp.tile([128, HPC * VW], F32R, tag=f"v{kb}",
                                        name=f"v{kb}"))

                # ---------------- phase 1: projections ----------------
                with contextlib.ExitStack() as p1:
                    xtp = p1.enter_context(tc.tile_pool(name="xt", bufs=1))
                    xt_sb = []
                    for ci in range(CI):
                        t_ = xtp.tile([128, T], F32R, tag=f"xt{ci}")
                        nc.sync.dma_start(t_[:],
                                          xT_d.ap()[ci * 128:(ci + 1) * 128, :])
                        xt_sb.append(t_)

                    # --- 1a: V (resident V_aug tiles) ---
                    with contextlib.ExitStack() as p1a:
                        wvp = p1a.enter_context(tc.tile_pool(name="wv", bufs=1))
                        vps = p1a.enter_context(
                            tc.tile_pool(name="vps", bufs=4, space="PSUM"))
                        wv_sb = []
                        for ci in range(CI):
                            t_ = wvp.tile([128, 512], F32R, tag=f"wv{ci}")
                            nc.sync.dma_start(
                                t_[:], wvT_d.ap()[ci * 128:(ci + 1) * 128, :])
                            wv_sb.append(t_)
                        for ti in range(KB):
                            ps_ = vps.tile([128, 512], F32)
                            for ci in range(CI):
                                nc.tensor.matmul(
                                    ps_[:],
                                    xt_sb[ci][:, ti * 128:(ti + 1) * 128],
                                    wv_sb[ci][:],
                                    start=(ci == 0), stop=(ci == CI - 1),
                                    skip_group_check=True)
                            sv = v_sb[ti][:].rearrange("p (h w) -> p h w", w=VW)
                            nc.vector.tensor_copy(
                                sv[:, :, 64:66],
                                ones16_r[:].rearrange("p (h w) -> p h w", w=2))
                            nc.vector.tensor_copy(
                                sv[:, :, 0:64],
                                ps_[:].rearrange("p (h w) -> p h w", w=64))

                    # --- 1b: Q^T, K^T (per head pair) ---
                    with contextlib.ExitStack() as p1b:
                        wqp = p1b.enter_context(tc.tile_pool(name="wq", bufs=2))
                        wkp = p1b.enter_context(tc.tile_pool(name="wk", bufs=2))
                        kstg = p1b.enter_context(tc.tile_pool(name="kstg", bufs=4))
                        qkps = p1b.enter_context(
                            tc.tile_pool(name="qkps", bufs=3, space="PSUM"))
                        for hp in range(HP):
                            fsl = slice(hp * 128, (hp + 1) * 128)
                            wq_sb, wk_sb = [], []
                            for ci in range(CI):
                                tq = wqp.tile([128, 128], F32R, tag=f"wqs{ci}")
                                nc.sync.dma_start(
                                    tq[:], wqT_d.ap()[ci * 128:(ci + 1) * 128, fsl])
                                wq_sb.append(tq)
                                tk = wkp.tile([128, 128], F32R, tag=f"wks{ci}")
                                nc.sync.dma_start(
                                    tk[:], wkT_d.ap()[ci * 128:(ci + 1) * 128, fsl])
                                wk_sb.append(tk)
                            for tj in range(NQSB):
                                tsl = slice(tj * 512, (tj + 1) * 512)
                                ps_ = qkps.tile([128, 512], F32)
                                for ci in range(CI):
                                    nc.tensor.matmul(
                                        ps_[:], wq_sb[ci][:], xt_sb[ci][:, tsl],
                                        start=(ci == 0), stop=(ci == CI - 1),
                                        skip_group_check=True)
                                nc.scalar.copy(qt_sb[hp][:, tsl], ps_[:])
                                ps2 = qkps.tile([128, 512], F32, tag="psk")
                                for ci in range(CI):
                                    nc.tensor.matmul(
                                        ps2[:], wk_sb[ci][:], xt_sb[ci][:, tsl],
                                        start=(ci == 0), stop=(ci == CI - 1),
                                        skip_group_check=True)
                                stg = kstg.tile([128, 512], F32R)
                                nc.vector.tensor_copy(stg[:], ps2[:])
                                nc.sync.dma_start(kT_spill.ap()[fsl, tsl], stg[:])

                # ---------------- phase 2: attention ----------------
                with contextlib.ExitStack() as p2:
                    maskp = p2.enter_context(tc.tile_pool(name="maskp", bufs=1))
                    ktp = p2.enter_context(tc.tile_pool(name="kt", bufs=1))
                    wop = p2.enter_context(tc.tile_pool(name="wo", bufs=1))
                    ptp = p2.enter_context(tc.tile_pool(name="pt", bufs=5))
                    rawp = p2.enter_context(tc.tile_pool(name="raw", bufs=4))
                    rrowp = p2.enter_context(tc.tile_pool(name="rrow", bufs=3))
                    tmpp = p2.enter_context(tc.tile_pool(name="tmp", bufs=3))
                    sps = p2.enter_context(
                        tc.tile_pool(name="sps", bufs=2, space="PSUM"))
                    spds = p2.enter_context(
                        tc.tile_pool(name="spds", bufs=1, space="PSUM"))
                    pvps = p2.enter_context(
                        tc.tile_pool(name="pvps", bufs=2, space="PSUM"))
                    bcps = p2.enter_context(
                        tc.tile_pool(name="bcps", bufs=1, space="PSUM"))

                    mask_sb = maskp.tile([128, MW], F32R)
                    nc.sync.dma_start(mask_sb[:], mask_d.ap())
                    kt_sb = []
                    for hp in range(HP):
                        k_ = ktp.tile([128, T], F32R, tag=f"kt{hp}",
                                      name=f"kt{hp}")
                        nc.sync.dma_start(
                            k_[:], kT_spill.ap()[hp * 128:(hp + 1) * 128, :])
                        kt_sb.append(k_)
                    wo_sb = []
                    for hp in range(HP):
                        w_ = wop.tile([128, C], F32R, tag=f"wo{hp}",
                                      name=f"wo{hp}")
                        nc.sync.dma_start(
                            w_[:], woT_d.ap()[hp * 128:(hp + 1) * 128, :])
                        wo_sb.append(w_)

                    for h in range(HPC):
                        hp, hl = h // 2, h % 2
                        psl = slice(hl * 64, hl * 64 + 64)
                        vsl = slice(h * VW, h * VW + 65)
                        for qsb in range(NQSB):
                            qbase = qsb * 512
                            n_full = 4 * qsb
                            pv = pvps.tile([128, 512], F32, tag="pv")
                            first = True
                            for kbp in range(n_full // 2):
                                kb0, kb1 = 2 * kbp, 2 * kbp + 1
                                sp_ = sps.tile([128, 1024], F32, tag="sp")
                                for u, kb in enumerate((kb0, kb1)):
                                    nc.tensor.matmul(
                                        sp_[:, u * 512:(u + 1) * 512],
                                        kt_sb[hp][psl, kb * 128:(kb + 1) * 128],
                                        qt_sb[hp][psl, qbase:qbase + 512],
                                        start=True, stop=True,
                                        skip_group_check=True)
                                pt = ptp.tile([128, MW], F32R, tag="pt")
                                nc.scalar.activation(pt[:, 0:1024], sp_[:],
                                                     EXP, scale=0.125)
                                for u, kb in enumerate((kb0, kb1)):
                                    nc.tensor.matmul(
                                        pv[0:65, :], v_sb[kb][:, vsl],
                                        pt[:, u * 512:(u + 1) * 512],
                                        start=first, stop=False,
                                        skip_group_check=True)
                                    first = False
                            # diagonal staircase: j0,j1 packed in a 2-bank
                            # psum tile, j2,j3 in a 1-bank tile; gapless
                            sp_a = sps.tile([128, 1024], F32, tag="sp")
                            sp_b = spds.tile([128, 512], F32, tag="spd")
                            diag_dst = (
                                (sp_a, 0), (sp_a, 512), (sp_b, 0), (sp_b, 256))
                            for j in range(4):
                                kb = n_full + j
                                n_ = 512 - QOFF[j]
                                dst, o_ = diag_dst[j]
                                nc.tensor.matmul(
                                    dst[:, o_:o_ + n_],
                                    kt_sb[hp][psl, kb * 128:(kb + 1) * 128],
                                    qt_sb[hp][psl,
                                              qbase + QOFF[j]:qbase + 512],
                                    start=True, stop=True,
                                    skip_group_check=True)
                            pt = ptp.tile([128, MW], F32R, tag="pt")
                            nc.scalar.activation(pt[:, 0:896], sp_a[:, 0:896],
                                                 EXP, scale=0.125)
                            nc.vector.tensor_mul(pt[:, 0:896], pt[:, 0:896],
                                                 mask_sb[:, 0:896])
                            nc.scalar.activation(pt[:, 896:MW], sp_b[:],
                                                 EXP, scale=0.125)
                            nc.vector.tensor_mul(pt[:, 896:MW], pt[:, 896:MW],
                                                 mask_sb[:, 896:MW])
                            for j in range(4):
                                kb = n_full + j
                                n_ = 512 - QOFF[j]
                                nc.tensor.matmul(
                                    pv[0:65, QOFF[j]:512], v_sb[kb][:, vsl],
                                    pt[:, POFF[j]:POFF[j] + n_],
                                    start=first, stop=(j == 3),
                                    skip_group_check=True)
                                first = False
                            # normalize: ctx = raw[0:64] / raw[64]
                            raw = rawp.tile([65, 512], F32)
                            nc.vector.tensor_copy(raw[:], pv[0:65, :])
                            rrow = rrowp.tile([65, 512], F32R)
                            with nc.allow_low_precision("softmax denom f32r"):
                                nc.vector.reciprocal(rrow[64:65, :],
                                                     raw[64:65, :])
                            bc = bcps.tile([64, 512], F32)
                            nc.tensor.matmul(bc[:], ones_r[64:65, :],
                                             rrow[64:65, :],
                                             start=True, stop=True,
                                             skip_group_check=True)
                            if hl == 0:
                                nc.vector.tensor_mul(
                                    ctx_sb[hp][0:64, qbase:qbase + 512],
                                    raw[0:64, :], bc[:])
                            else:
                                tmp = tmpp.tile([64, 512], F32R)
                                nc.vector.tensor_mul(tmp[:], raw[0:64, :],
                                                     bc[:])
                                nc.sync.dma_start(
                                    ctx_sb[hp][64:128, qbase:qbase + 512],
                                    tmp[:])

                    # -------------- phase 3: output projection --------------
                    with contextlib.ExitStack() as p3:
                        yp = p3.enter_context(tc.tile_pool(name="y", bufs=3))
                        for oi in range(8):
                            osl = slice(oi * 128, (oi + 1) * 128)
                            for tj in range(NQSB):
                                tsl = slice(tj * 512, (tj + 1) * 512)
                                ps_ = pvps.tile([128, 512], F32, tag="pv",
                                                name="yacc")
                                for hp in range(HP):
                                    nc.tensor.matmul(
                                        ps_[:], wo_sb[hp][:, osl],
                                        ctx_sb[hp][:, tsl],
                                        start=(hp == 0), stop=(hp == HP - 1),
                                        skip_group_check=True)
                                y_ = yp.tile([128, 512], F32)
                                nc.vector.tensor_scalar_add(
                                    y_[:], ps_[:], bias_sb[:, oi:oi + 1])
                                nc.sync.dma_start(yT_d.ap()[osl, tsl], y_[:])

        if iters == 1:
            emit()
        else:
            with tc.For_i(0, iters, 1):
                emit()
    nc.compile()
    return nc


def make_masks():
    """Merged staircase mask [128, MW]: psum col POFF[j] + (q - QOFF[j])
    holds causal keep-bit for key row k = 128*j + k_local vs query q."""
    m = np.zeros((128, MW), np.float32)
    k = np.arange(128)[:, None]
    for j in range(4):
        q = np.arange(QOFF[j], 512)[None, :]
        m[:, POFF[j]:POFF[j] + 512 - QOFF[j]] = (q >= 128 * j + k)
    return m


def shard_inputs(x, w_qkv, w_out, b_out):
    """Full inputs -> list of 8 per-core input dicts."""
    x = np.asarray(x, dtype=np.float32)
    w_qkv = np.asarray(w_qkv, dtype=np.float32)
    w_out = np.asarray(w_out, dtype=np.float32)
    b_out = np.asarray(b_out, dtype=np.float32)
    masks = make_masks()
    in_maps = []
    for c in range(N_CORES):
        b, hg = c // 2, c % 2
        h0 = hg * HPC
        csl = slice(h0 * D, (h0 + HPC) * D)
        im = {
            "xT": np.ascontiguousarray(x[b].T),
            "wqT": np.ascontiguousarray(w_qkv[0 * C:1 * C][csl].T),
            "wkT": np.ascontiguousarray(w_qkv[1 * C:2 * C][csl].T),
            "wvT": np.ascontiguousarray(w_qkv[2 * C:3 * C][csl].T),
            "woT": np.ascontiguousarray(w_out[:, csl].T),
            "bias": (np.ascontiguousarray(b_out.reshape(8, 128).T)
                     if hg == 0 else np.zeros((128, 8), np.float32)),
            "masks": masks,
        }
        in_maps.append(im)
    return in_maps


def gather_outputs(results):
    """8 per-core {'yT': [C,T]} -> full [B,T,C]."""
    y = np.empty((B, T, C), np.float32)
    for b in range(B):
        acc = results[2 * b]["yT"] + results[2 * b + 1]["yT"]
        y[b] = acc.T
    return y


def kernel(**inputs):
    from concourse.bass_utils import run_bass_kernel_spmd
    if "nc" not in _CACHE:
        _CACHE["nc"] = build_nc()
    nc = _CACHE["nc"]
    in_maps = shard_inputs(inputs["x"], inputs["w_qkv"],
                           inputs["w_out"], inputs["b_out"])
    res = run_bass_kernel_spmd(nc, in_maps, list(range(N_CORES)))
    return gather_outputs(res.results)



# revision 6
# speedup vs baseline: 1.2496x; 1.2496x over previous
"""Causal multi-head attention (B=4, T=2048, C=1024, H=16) on 8 TRN2 cores.

Sharding: batch (4) x head-group (2 groups of 8 heads) -> 8 shards, one per
core. Each core computes QKV projections for its 8 heads, causal flash-style
attention, and a Megatron row-parallel slice of the output projection; the
host sums the two head-group partial outputs per batch element.

All matmul operands are bf16 (PE streams at 2.4 GHz vs fp32r's 1.2), PSUM
accumulation stays f32. K is kept resident in SBUF (no DRAM spill). Q/K
projections are emitted per head-pair between attention blocks so ScalarE's
exp work overlaps PE projection work. The two heads of a pair occupy PE
row-bands 0-63 / 64-127, so their S^T matmuls run concurrently (row-group
tiling via base_partition). Softmax denominators use the fast approximate
reciprocal; the diagonal staircase is exact (widths 512/384/256/128).

Per-core dataflow:
  V   = xT c-tiles (lhsT) @ wvT -> [t,dv] -> resident V_aug tiles (+ones row)
  Q^T,K^T = wq/wkT (lhsT) @ xT -> [f,t] resident
  per (head-pair, 512-query block): S^T = K^T.T @ Q^T per 128-k tile,
  P^T = exp(S^T/8) (ACT; staircase mask multiply on DVE), PV^T accumulated
  with V_aug stationary -> [d+1, q] (row 64 = l), normalize via fast recip +
  PE ones-broadcast -> ctx^T [c,t];  y^T = woT (lhsT) @ ctx^T + bias -> DRAM

Self-contained: hardcodes shapes from the problem spec; no file reads.
"""
import sys
sys.path.insert(0, '/opt/trn_rl_repo')
import numpy as np

B, T, C = 4, 2048, 1024
H, D = 16, 64
N_CORES = 8
HPC = 8        # heads per core
HP = 4         # head pairs per core
KB = 16        # 128-row key tiles per sequence
NQSB = 4       # 512-column query superblocks
CI = 8         # 128-row contraction tiles over C
VW = 66        # V_aug stride per head (64 V + 1 ones + 1 pad)

# Exact diagonal staircase inside two psum tiles per head:
#   tile A [128,1024]: j0 (512 q-cols) at 0, j1 (384) at 512
#   tile B [128, 384]: j2 (256) at 0,        j3 (128) at 256
# QOFF[j] = first query column covered by diag block j; FOFF[j] = flat mask col.
QOFF = (0, 128, 256, 384)
FOFF = (0, 512, 896, 1152)
MW = 1280      # total mask width

_CACHE = {}


def build_nc(iters=1):
    import contextlib
    import concourse.tile as tile
    from concourse import bacc, mybir

    F32 = mybir.dt.float32
    F32R = mybir.dt.float32r
    BF16 = mybir.dt.bfloat16
    EXP = mybir.ActivationFunctionType.Exp

    nc = bacc.Bacc("TRN2", target_bir_lowering=False, debug=False)

    xT_d = nc.dram_tensor("xT", [C, T], BF16, kind="ExternalInput")
    wqT_d = nc.dram_tensor("wqT", [C, 512], BF16, kind="ExternalInput")
    wkT_d = nc.dram_tensor("wkT", [C, 512], BF16, kind="ExternalInput")
    wvT_d = nc.dram_tensor("wvT", [C, 512], BF16, kind="ExternalInput")
    woT_d = nc.dram_tensor("woT", [512, C], BF16, kind="ExternalInput")
    bias_d = nc.dram_tensor("bias", [128, 8], F32, kind="ExternalInput")
    mask_d = nc.dram_tensor("masks", [128, MW], BF16, kind="ExternalInput")
    yT_d = nc.dram_tensor("yT", [C, T], F32, kind="ExternalOutput")

    with tile.TileContext(nc) as tc:
        def emit():
            with contextlib.ExitStack() as es:
                const = es.enter_context(tc.tile_pool(name="const", bufs=1))
                qtp = es.enter_context(tc.tile_pool(name="qt", bufs=1))
                ktp = es.enter_context(tc.tile_pool(name="kt", bufs=1))
                ctxp = es.enter_context(tc.tile_pool(name="ctx", bufs=1))
                vp = es.enter_context(tc.tile_pool(name="vsb", bufs=1))
                xtp = es.enter_context(tc.tile_pool(name="xt", bufs=1))
                wvp = es.enter_context(tc.tile_pool(name="wv", bufs=1))
                wqp = es.enter_context(tc.tile_pool(name="wq", bufs=2))
                wkp = es.enter_context(tc.tile_pool(name="wk", bufs=2))
                wop = es.enter_context(tc.tile_pool(name="wo", bufs=1))
                maskp = es.enter_context(tc.tile_pool(name="maskp", bufs=1))
                ptp = es.enter_context(tc.tile_pool(name="pt", bufs=4))
                rawp = es.enter_context(tc.tile_pool(name="raw", bufs=3))
                rrowp = es.enter_context(tc.tile_pool(name="rrow", bufs=2))
                tmpp = es.enter_context(tc.tile_pool(name="tmp", bufs=2))
                yp = es.enter_context(tc.tile_pool(name="y", bufs=3))
                # PSUM: "sp" 2x[128,1024] = 4 banks; "pv" 4x[128,512] = 4.
                sps = es.enter_context(
                    tc.tile_pool(name="sps", bufs=2, space="PSUM"))
                pps = es.enter_context(
                    tc.tile_pool(name="pps", bufs=4, space="PSUM"))

                ones_f = const.tile([128, 64], F32)
                nc.any.memset(ones_f[:], 1.0)
                ones_r = const.tile([128, 64], F32R)
                nc.vector.tensor_copy(ones_r[:], ones_f[:])
                ones16_b = const.tile([128, 16], BF16)
                nc.vector.tensor_copy(ones16_b[:], ones_f[:, 0:16])
                bias_sb = const.tile([128, 8], F32)
                nc.sync.dma_start(bias_sb[:], bias_d.ap())
                mask_sb = maskp.tile([128, MW], BF16)
                nc.sync.dma_start(mask_sb[:], mask_d.ap())

                qt_sb, kt_sb, ctx_sb, v_sb = [], [], [], []
                for hp in range(HP):
                    qt_sb.append(qtp.tile([128, T], BF16, tag=f"qt{hp}",
                                          name=f"qt{hp}"))
                    kt_sb.append(ktp.tile([128, T], BF16, tag=f"kt{hp}",
                                          name=f"kt{hp}"))
                    ctx_sb.append(ctxp.tile([128, T], BF16, tag=f"ctx{hp}",
                                            name=f"ctx{hp}"))
                for kb in range(KB):
                    v_sb.append(vp.tile([128, HPC * VW], BF16, tag=f"v{kb}",
                                        name=f"v{kb}"))

                xt_sb = []
                for ci in range(CI):
                    t_ = xtp.tile([128, T], BF16, tag=f"xt{ci}")
                    nc.sync.dma_start(t_[:],
                                      xT_d.ap()[ci * 128:(ci + 1) * 128, :])
                    xt_sb.append(t_)
                wo_sb = []
                for hp in range(HP):
                    w_ = wop.tile([128, C], BF16, tag=f"wo{hp}",
                                  name=f"wo{hp}")
                    nc.sync.dma_start(
                        w_[:], woT_d.ap()[hp * 128:(hp + 1) * 128, :])
                    wo_sb.append(w_)

                # ---------------- V projection (resident V_aug) ----------
                wv_sb = []
                for ci in range(CI):
                    t_ = wvp.tile([128, 512], BF16, tag=f"wv{ci}")
                    nc.sync.dma_start(
                        t_[:], wvT_d.ap()[ci * 128:(ci + 1) * 128, :])
                    wv_sb.append(t_)
                for ti in range(KB):
                    ps_ = pps.tile([128, 512], F32, tag="pv", name="vps")
                    for ci in range(CI):
                        nc.tensor.matmul(
                            ps_[:],
                            xt_sb[ci][:, ti * 128:(ti + 1) * 128],
                            wv_sb[ci][:],
                            start=(ci == 0), stop=(ci == CI - 1),
                            skip_group_check=True)
                    sv = v_sb[ti][:].rearrange("p (h w) -> p h w", w=VW)
                    nc.vector.tensor_copy(
                        sv[:, :, 64:66],
                        ones16_b[:].rearrange("p (h w) -> p h w", w=2))
                    nc.vector.tensor_copy(
                        sv[:, :, 0:64],
                        ps_[:].rearrange("p (h w) -> p h w", w=64))

                def project_qk(hp):
                    fsl = slice(hp * 128, (hp + 1) * 128)
                    wq_sb, wk_sb = [], []
                    for ci in range(CI):
                        tq = wqp.tile([128, 128], BF16, tag=f"wqs{ci}")
                        nc.sync.dma_start(
                            tq[:], wqT_d.ap()[ci * 128:(ci + 1) * 128, fsl])
                        wq_sb.append(tq)
                        tk = wkp.tile([128, 128], BF16, tag=f"wks{ci}")
                        nc.sync.dma_start(
                            tk[:], wkT_d.ap()[ci * 128:(ci + 1) * 128, fsl])
                        wk_sb.append(tk)
                    for tj in range(NQSB):
                        tsl = slice(tj * 512, (tj + 1) * 512)
                        ps_ = pps.tile([128, 512], F32, tag="pv", name="qps")
                        for ci in range(CI):
                            nc.tensor.matmul(
                                ps_[:], wq_sb[ci][:], xt_sb[ci][:, tsl],
                                start=(ci == 0), stop=(ci == CI - 1),
                                skip_group_check=True)
                        nc.scalar.copy(qt_sb[hp][:, tsl], ps_[:])
                        ps2 = pps.tile([128, 512], F32, tag="pv", name="kps")
                        for ci in range(CI):
                            nc.tensor.matmul(
                                ps2[:], wk_sb[ci][:], xt_sb[ci][:, tsl],
                                start=(ci == 0), stop=(ci == CI - 1),
                                skip_group_check=True)
                        nc.vector.tensor_copy(kt_sb[hp][:, tsl], ps2[:])

                def attention_pair(hp, qsb):
                    psl = (slice(0, 64), slice(64, 128))
                    vsl = (slice((2 * hp) * VW, (2 * hp) * VW + 65),
                           slice((2 * hp + 1) * VW, (2 * hp + 1) * VW + 65))
                    qbase = qsb * 512
                    n_full = 4 * qsb
                    pv = [pps.tile([128, 512], F32, tag="pv", name="pv0"),
                          pps.tile([128, 512], F32, tag="pv", name="pv1")]
                    first = [True, True]

                    def pv_mm(hl, kb, pt_ap, q0, q1, stop=False):
                        nc.tensor.matmul(
                            pv[hl][0:65, q0:q1], v_sb[kb][:, vsl[hl]], pt_ap,
                            start=first[hl], stop=stop,
                            skip_group_check=True)
                        first[hl] = False

                    for kbp in range(n_full // 2):
                        kb0, kb1 = 2 * kbp, 2 * kbp + 1
                        sp = [sps.tile([128, 1024], F32, tag="sp", name="sp0"),
                              sps.tile([128, 1024], F32, tag="sp", name="sp1")]
                        for u, kb in enumerate((kb0, kb1)):
                            for hl in range(2):
                                nc.tensor.matmul(
                                    sp[hl][:, u * 512:(u + 1) * 512],
                                    kt_sb[hp][psl[hl], kb * 128:(kb + 1) * 128],
                                    qt_sb[hp][psl[hl], qbase:qbase + 512],
                                    start=True, stop=True,
                                    skip_group_check=True)
                        for hl in range(2):
                            pt = ptp.tile([128, 1024], BF16, tag="pt",
                                          name="pt")
                            nc.scalar.activation(pt[:], sp[hl][:],
                                                 EXP, scale=0.125)
                            pv_mm(hl, kb0, pt[:, 0:512], 0, 512)
                            pv_mm(hl, kb1, pt[:, 512:1024], 0, 512)

                    # exact diagonal staircase
                    spa = [sps.tile([128, 1024], F32, tag="sp", name="spa0"),
                           sps.tile([128, 1024], F32, tag="sp", name="spa1")]
                    for u, j in enumerate((0, 1)):
                        kb = n_full + j
                        for hl in range(2):
                            nc.tensor.matmul(
                                spa[hl][:, u * 512:u * 512 + 512 - QOFF[j]],
                                kt_sb[hp][psl[hl], kb * 128:(kb + 1) * 128],
                                qt_sb[hp][psl[hl],
                                          qbase + QOFF[j]:qbase + 512],
                                start=True, stop=True,
                                skip_group_check=True)
                    pta = []
                    for hl in range(2):
                        pt = ptp.tile([128, 896], BF16, tag="pt", name="pta")
                        nc.scalar.activation(pt[:], spa[hl][:, 0:896],
                                             EXP, scale=0.125)
                        nc.vector.tensor_mul(pt[:], pt[:], mask_sb[:, 0:896])
                        pta.append(pt)
                    spb = [sps.tile([128, 384], F32, tag="sp", name="spb0"),
                           sps.tile([128, 384], F32, tag="sp", name="spb1")]
                    for u, j in enumerate((2, 3)):
                        kb = n_full + j
                        o_ = (0, 256)[u]
                        for hl in range(2):
                            nc.tensor.matmul(
                                spb[hl][:, o_:o_ + 512 - QOFF[j]],
                                kt_sb[hp][psl[hl], kb * 128:(kb + 1) * 128],
                                qt_sb[hp][psl[hl],
                                          qbase + QOFF[j]:qbase + 512],
                                start=True, stop=True,
                                skip_group_check=True)
                    ptb = []
                    for hl in range(2):
                        pt = ptp.tile([128, 384], BF16, tag="pt", name="ptb")
                        nc.scalar.activation(pt[:], spb[hl][:],
                                             EXP, scale=0.125)
                        nc.vector.tensor_mul(pt[:], pt[:],
                                             mask_sb[:, 896:MW])
                        ptb.append(pt)
                    for hl in range(2):
                        pv_mm(hl, n_full + 0, pta[hl][:, 0:512], 0, 512)
                        pv_mm(hl, n_full + 1, pta[hl][:, 512:896], 128, 512)
                        pv_mm(hl, n_full + 2, ptb[hl][:, 0:256], 256, 512)
                        pv_mm(hl, n_full + 3, ptb[hl][:, 256:384], 384, 512,
                              stop=True)

                    # normalize: ctx = pv[0:64] * (1 / pv[64])
                    for hl in range(2):
                        raw = rawp.tile([65, 512], F32, name="raw")
                        nc.vector.tensor_copy(raw[:], pv[hl][0:65, :])
                        # custom DVE ops no-op at nonzero base partition:
                        # run over all 65 rows (same cost, lanes parallel)
                        rrow_f = rrowp.tile([65, 512], F32, tag="rf",
                                            name="rrow_f")
                        nc.vector.reciprocal_approx_fast(
                            rrow_f[0:65, :], raw[0:65, :])
                        rrow = rrowp.tile([65, 512], F32R, tag="rr",
                                          name="rrow")
                        nc.scalar.copy(rrow[0:65, :], rrow_f[0:65, :])
                        bc = sps.tile([64, 512], F32, tag="sp", name="bc")
                        nc.tensor.matmul(bc[:], ones_r[64:65, :],
                                         rrow[64:65, :],
                                         start=True, stop=True,
                                         skip_group_check=True)
                        if hl == 0:
                            nc.vector.tensor_mul(
                                ctx_sb[hp][0:64, qbase:qbase + 512],
                                raw[0:64, :], bc[:])
                        else:
                            tmp = tmpp.tile([64, 512], BF16, name="tmp")
                            nc.vector.tensor_mul(tmp[:], raw[0:64, :],
                                                 bc[:])
                            nc.sync.dma_start(
                                ctx_sb[hp][64:128, qbase:qbase + 512],
                                tmp[:])

                # interleave projections with attention per head pair
                for hp in range(HP):
                    project_qk(hp)
                    for qsb in range(NQSB):
                        attention_pair(hp, qsb)

                # ---------------- output projection ----------------
                for oi in range(8):
                    osl = slice(oi * 128, (oi + 1) * 128)
                    for tj in range(NQSB):
                        tsl = slice(tj * 512, (tj + 1) * 512)
                        ps_ = pps.tile([128, 512], F32, tag="pv",
                                       name="yacc")
                        for hp in range(HP):
                            nc.tensor.matmul(
                                ps_[:], wo_sb[hp][:, osl],
                                ctx_sb[hp][:, tsl],
                                start=(hp == 0), stop=(hp == HP - 1),
                                skip_group_check=True)
                        y_ = yp.tile([128, 512], F32)
                        nc.vector.tensor_scalar_add(
                            y_[:], ps_[:], bias_sb[:, oi:oi + 1])
                        nc.sync.dma_start(yT_d.ap()[osl, tsl], y_[:])

        if iters == 1:
            emit()
        else:
            with tc.For_i(0, iters, 1):
                emit()
    nc.compile()
    return nc


def make_masks():
    """Exact staircase mask [128, MW]: flat col FOFF[j] + (q - QOFF[j])
    holds causal keep-bit for key row k = 128*j + k_local vs query q."""
    m = np.zeros((128, MW), np.float32)
    k = np.arange(128)[:, None]
    for j in range(4):
        q = np.arange(QOFF[j], 512)[None, :]
        m[:, FOFF[j]:FOFF[j] + 512 - QOFF[j]] = (q >= 128 * j + k)
    return m


def shard_inputs(x, w_qkv, w_out, b_out):
    """Full inputs -> list of 8 per-core input dicts (weights/x in bf16)."""
    import ml_dtypes
    bf16 = ml_dtypes.bfloat16
    x = np.asarray(x, dtype=np.float32)
    w_qkv = np.asarray(w_qkv, dtype=np.float32)
    w_out = np.asarray(w_out, dtype=np.float32)
    b_out = np.asarray(b_out, dtype=np.float32)
    masks = make_masks().astype(bf16)
    in_maps = []
    for c in range(N_CORES):
        b, hg = c // 2, c % 2
        h0 = hg * HPC
        csl = slice(h0 * D, (h0 + HPC) * D)
        im = {
            "xT": np.ascontiguousarray(x[b].T).astype(bf16),
            "wqT": np.ascontiguousarray(w_qkv[0 * C:1 * C][csl].T).astype(bf16),
            "wkT": np.ascontiguousarray(w_qkv[1 * C:2 * C][csl].T).astype(bf16),
            "wvT": np.ascontiguousarray(w_qkv[2 * C:3 * C][csl].T).astype(bf16),
            "woT": np.ascontiguousarray(w_out[:, csl].T).astype(bf16),
            "bias": (np.ascontiguousarray(b_out.reshape(8, 128).T)
                     if hg == 0 else np.zeros((128, 8), np.float32)),
            "masks": masks,
        }
        in_maps.append(im)
    return in_maps


def gather_outputs(results):
    """8 per-core {'yT': [C,T]} -> full [B,T,C]."""
    y = np.empty((B, T, C), np.float32)
    for b in range(B):
        acc = results[2 * b]["yT"] + results[2 * b + 1]["yT"]
        y[b] = acc.T
    return y


def kernel(**inputs):
    from concourse.bass_utils import run_bass_kernel_spmd
    if "nc" not in _CACHE:
        _CACHE["nc"] = build_nc()
    nc = _CACHE["nc"]
    in_maps = shard_inputs(inputs["x"], inputs["w_qkv"],
                           inputs["w_out"], inputs["b_out"])
    res = run_bass_kernel_spmd(nc, in_maps, list(range(N_CORES)))
    return gather_outputs(res.results)


# revision 16
# speedup vs baseline: 1.7164x; 1.3735x over previous
"""Causal multi-head attention (B=4, T=2048, C=1024, H=16) on 8 TRN2 cores.

Sharding: batch (4) x head-group (2 groups of 8 heads) -> 8 shards, one per
core. Each core computes QKV projections for its 8 heads, causal flash-style
attention, and a Megatron row-parallel slice of the output projection; the
host sums the two head-group partial outputs per batch element.

All matmul operands are bf16 (PE streams at 2.4 GHz vs fp32r's 1.2), PSUM
accumulation stays f32. K stays resident in SBUF (no DRAM spill).

Attention is emitted as a software pipeline over 128-key slots. Each slot's
S^T tile packs BOTH heads of a pair side by side ([128, 2w]: head hl=0 in
cols 0:w from PE rows 0-63, hl=1 in cols w:2w from rows 64-127): the two
matmuls land in different PSUM banks and different PE row-groups, so they
run concurrently, and one ACT exp covers both heads. PV for slot k is
emitted after S of slot k+1 (lag 1) so the PE isn't gated on ScalarE's exp.
Q/K projection matmul groups for the NEXT head pair are interleaved into the
attention stream as filler to keep the PE dense while ScalarE drains exp;
the output projection for query block tj rides behind the last head pair's
attention on that block. Softmax normalization (1/l) runs entirely off the
PE: DVE copies release PSUM, then fast-reciprocal + GpSimd
partition_broadcast + DVE multiply produce ctx.

Self-contained: hardcodes shapes from the problem spec; no file reads.
"""
import sys
sys.path.insert(0, '/opt/trn_rl_repo')
import numpy as np

B, T, C = 4, 2048, 1024
H, D = 16, 64
N_CORES = 8
HPC = 8        # heads per core
HP = 4         # head pairs per core
KB = 16        # 128-row key tiles per sequence
NQSB = 4       # 512-column query superblocks
CI = 8         # 128-row contraction tiles over C
VW = 66        # V_aug stride per head (64 V + 1 ones + 1 pad)

# Diagonal staircase: block j covers queries [QOFF[j], 512) of the
# superblock (widths 512/384/256/128, exact causal trim at 128 granularity).
# Mask tile layout matches the packed psum tiles: A = j0|j0 (cols 0:1024),
# B = j1,j3|j1,j3 (1024:2048), C = j2 (2048:2304).
QOFF = (0, 128, 256, 384)
DW = tuple(512 - q for q in QOFF)
MW2 = 2304

_CACHE = {}


def build_nc(iters=1):
    import contextlib
    import concourse.tile as tile
    from concourse import bacc, mybir

    F32 = mybir.dt.float32
    BF16 = mybir.dt.bfloat16
    EXP = mybir.ActivationFunctionType.Exp

    nc = bacc.Bacc("TRN2", target_bir_lowering=False, debug=False)

    xT_d = nc.dram_tensor("xT", [C, T], BF16, kind="ExternalInput")
    wqT_d = nc.dram_tensor("wqT", [C, 512], BF16, kind="ExternalInput")
    wkT_d = nc.dram_tensor("wkT", [C, 512], BF16, kind="ExternalInput")
    wvT_d = nc.dram_tensor("wvT", [C, 512], BF16, kind="ExternalInput")
    woT_d = nc.dram_tensor("woT", [512, C], BF16, kind="ExternalInput")
    bias_d = nc.dram_tensor("bias", [128, 8], F32, kind="ExternalInput")
    mask_d = nc.dram_tensor("masks", [128, MW2], BF16, kind="ExternalInput")
    yT_d = nc.dram_tensor("yT", [C, T], F32, kind="ExternalOutput")

    with tile.TileContext(nc) as tc:
        with contextlib.ExitStack() as es:
            # Pools live outside the For_i loop; tag rotation carries
            # cross-iteration dependencies.
            const = es.enter_context(tc.tile_pool(name="const", bufs=1))
            qtp = es.enter_context(tc.tile_pool(name="qt", bufs=1))
            ktp = es.enter_context(tc.tile_pool(name="kt", bufs=1))
            ctxp = es.enter_context(tc.tile_pool(name="ctx", bufs=1))
            vp = es.enter_context(tc.tile_pool(name="vsb", bufs=1))
            xtp = es.enter_context(tc.tile_pool(name="xt", bufs=1))
            wvp = es.enter_context(tc.tile_pool(name="wv", bufs=1))
            wqp = es.enter_context(tc.tile_pool(name="wq", bufs=2))
            wkp = es.enter_context(tc.tile_pool(name="wk", bufs=2))
            wop = es.enter_context(tc.tile_pool(name="wo", bufs=1))
            maskp = es.enter_context(tc.tile_pool(name="maskp", bufs=1))
            ptp = es.enter_context(tc.tile_pool(name="pt", bufs=4))
            rawp = es.enter_context(tc.tile_pool(name="raw", bufs=3))
            rrowp = es.enter_context(tc.tile_pool(name="rrow", bufs=3))
            bcp = es.enter_context(tc.tile_pool(name="bcp", bufs=3))
            tmpp = es.enter_context(tc.tile_pool(name="tmp", bufs=2))
            yp = es.enter_context(tc.tile_pool(name="y", bufs=3))
            # PSUM: "sp" 2x[128,1024] = 4 banks; "pj" (projections, own
            # tag so filler groups never wait on open pv accumulations)
            # 2x[128,512] = 2; "pv" (attention accumulators) 2x[128,512] = 2.
            sps = es.enter_context(
                tc.tile_pool(name="sps", bufs=2, space="PSUM"))
            pps = es.enter_context(
                tc.tile_pool(name="pps", bufs=2, space="PSUM"))

            def emit():
                ones_f = const.tile([128, 64], F32)
                nc.any.memset(ones_f[:], 1.0)
                ones16_b = const.tile([128, 16], BF16)
                nc.vector.tensor_copy(ones16_b[:], ones_f[:, 0:16])
                bias_sb = const.tile([128, 8], F32)
                nc.sync.dma_start(bias_sb[:], bias_d.ap())
                mask_sb = maskp.tile([128, MW2], BF16)
                nc.sync.dma_start(mask_sb[:], mask_d.ap())

                qt_sb, kt_sb, ctx_sb, v_sb = [], [], [], []
                for hp in range(HP):
                    qt_sb.append(qtp.tile([128, T], BF16, tag=f"qt{hp}",
                                          name=f"qt{hp}"))
                    kt_sb.append(ktp.tile([128, T], BF16, tag=f"kt{hp}",
                                          name=f"kt{hp}"))
                    ctx_sb.append(ctxp.tile([128, T], BF16, tag=f"ctx{hp}",
                                            name=f"ctx{hp}"))
                for kb in range(KB):
                    v_sb.append(vp.tile([128, HPC * VW], BF16, tag=f"v{kb}",
                                        name=f"v{kb}"))

                xt_sb = []
                for ci in range(CI):
                    t_ = xtp.tile([128, T], BF16, tag=f"xt{ci}")
                    nc.sync.dma_start(t_[:],
                                      xT_d.ap()[ci * 128:(ci + 1) * 128, :])
                    xt_sb.append(t_)
                wo_sb = []
                for hp in range(HP):
                    w_ = wop.tile([128, C], BF16, tag=f"wo{hp}",
                                  name=f"wo{hp}")
                    nc.sync.dma_start(
                        w_[:], woT_d.ap()[hp * 128:(hp + 1) * 128, :])
                    wo_sb.append(w_)

                # ---------------- V projection (resident V_aug) ----------
                wv_sb = []
                for ci in range(CI):
                    t_ = wvp.tile([128, 512], BF16, tag=f"wv{ci}")
                    nc.sync.dma_start(
                        t_[:], wvT_d.ap()[ci * 128:(ci + 1) * 128, :])
                    wv_sb.append(t_)
                for ti in range(KB):
                    ps_ = pps.tile([128, 512], F32, tag="pj", name="vps")
                    for ci in range(CI):
                        nc.tensor.matmul(
                            ps_[:],
                            xt_sb[ci][:, ti * 128:(ti + 1) * 128],
                            wv_sb[ci][:],
                            start=(ci == 0), stop=(ci == CI - 1),
                            skip_group_check=True)
                    sv = v_sb[ti][:].rearrange("p (h w) -> p h w", w=VW)
                    nc.vector.tensor_copy(
                        sv[:, :, 64:66],
                        ones16_b[:].rearrange("p (h w) -> p h w", w=2))
                    nc.vector.tensor_copy(
                        sv[:, :, 0:64],
                        ps_[:].rearrange("p (h w) -> p h w", w=64))

                def proj_group_fns(hp):
                    """8 filler callables: Q then K psum groups for pair hp."""
                    fsl = slice(hp * 128, (hp + 1) * 128)
                    wq_sb, wk_sb = [], []

                    def load_w():
                        for ci in range(CI):
                            tq = wqp.tile([128, 128], BF16, tag=f"wqs{ci}",
                                          name="wq")
                            nc.sync.dma_start(
                                tq[:],
                                wqT_d.ap()[ci * 128:(ci + 1) * 128, fsl])
                            wq_sb.append(tq)
                            tk = wkp.tile([128, 128], BF16, tag=f"wks{ci}",
                                          name="wk")
                            nc.sync.dma_start(
                                tk[:],
                                wkT_d.ap()[ci * 128:(ci + 1) * 128, fsl])
                            wk_sb.append(tk)

                    def qgroup(tj):
                        tsl = slice(tj * 512, (tj + 1) * 512)
                        ps_ = pps.tile([128, 512], F32, tag="pj", name="qps")
                        for ci in range(CI):
                            nc.tensor.matmul(
                                ps_[:], wq_sb[ci][:], xt_sb[ci][:, tsl],
                                start=(ci == 0), stop=(ci == CI - 1),
                                skip_group_check=True)
                        nc.scalar.copy(qt_sb[hp][:, tsl], ps_[:])

                    def kgroup(tj):
                        tsl = slice(tj * 512, (tj + 1) * 512)
                        ps_ = pps.tile([128, 512], F32, tag="pj", name="kps")
                        for ci in range(CI):
                            nc.tensor.matmul(
                                ps_[:], wk_sb[ci][:], xt_sb[ci][:, tsl],
                                start=(ci == 0), stop=(ci == CI - 1),
                                skip_group_check=True)
                        nc.vector.tensor_copy(kt_sb[hp][:, tsl], ps_[:])

                    fns = [load_w]
                    for tj in range(NQSB):
                        fns.append(lambda tj=tj: qgroup(tj))
                        fns.append(lambda tj=tj: kgroup(tj))
                    return fns

                def attention_pair(hp, qsb, filler):
                    psl = (slice(0, 64), slice(64, 128))
                    vsl = (slice((2 * hp) * VW, (2 * hp) * VW + 65),
                           slice((2 * hp + 1) * VW, (2 * hp + 1) * VW + 65))
                    qbase = qsb * 512
                    n_full = 4 * qsb
                    pv = [pps.tile([128, 512], F32, tag="pv", name="pv0"),
                          pps.tile([128, 512], F32, tag="pv", name="pv1")]
                    first = [True, True]
                    # Slots: n_full full-key tiles plus three diagonal tiles
                    # DA/DB/DC. Every matmul's PSUM output stays inside one
                    # bank (cols 0:512 / 512:1024), and the hl0/hl1 pair of
                    # each block targets different banks so the row-group
                    # concurrency is legal:
                    #   full/DA [128,1024]: hl0 j at 0:512, hl1 j at 512:1024
                    #   DB [128,1024]: j1 h0 0:384, j3 h0 384:512,
                    #                  j1 h1 512:896, j3 h1 896:1024
                    #   DC [128,768]:  j2 h0 0:256, j2 h1 512:768 (gap unread)
                    def smm(sp, c0, c1, hl, kb, qoff):
                        nc.tensor.matmul(
                            sp[:, c0:c1],
                            kt_sb[hp][psl[hl], kb * 128:(kb + 1) * 128],
                            qt_sb[hp][psl[hl], qbase + qoff:qbase + 512],
                            start=True, stop=True, skip_group_check=True)

                    def pvmm(hl, kb, pt_ap, qoff, stop):
                        nc.tensor.matmul(
                            pv[hl][0:65, qoff:512],
                            v_sb[kb][:, vsl[hl]], pt_ap,
                            start=first[hl], stop=stop,
                            skip_group_check=True)
                        first[hl] = False

                    nslots = n_full + 3
                    pts = {}

                    def emit_S(idx):
                        if idx < n_full + 1:          # full tile or DA
                            kb = idx
                            sp = sps.tile([128, 1024], F32, tag="sp",
                                          name="sp")
                            smm(sp, 0, 512, 0, kb, 0)
                            smm(sp, 512, 1024, 1, kb, 0)
                            pt = ptp.tile([128, 1024], BF16, tag="pt",
                                          name="pt")
                            nc.scalar.activation(pt[:], sp[:], EXP,
                                                 scale=0.125)
                            if idx == n_full:         # DA: j0 triangle mask
                                nc.vector.tensor_mul(pt[:], pt[:],
                                                     mask_sb[:, 0:1024])
                        elif idx == n_full + 1:       # DB: j1 + j3
                            sp = sps.tile([128, 1024], F32, tag="sp",
                                          name="sp")
                            smm(sp, 0, 384, 0, n_full + 1, 128)
                            smm(sp, 512, 896, 1, n_full + 1, 128)
                            smm(sp, 384, 512, 0, n_full + 3, 384)
                            smm(sp, 896, 1024, 1, n_full + 3, 384)
                            pt = ptp.tile([128, 1024], BF16, tag="pt",
                                          name="pt")
                            nc.scalar.activation(pt[:], sp[:], EXP,
                                                 scale=0.125)
                            nc.vector.tensor_mul(pt[:], pt[:],
                                                 mask_sb[:, 1024:2048])
                        else:                         # DC: j2
                            sp = sps.tile([128, 768], F32, tag="sp",
                                          name="sp")
                            smm(sp, 0, 256, 0, n_full + 2, 256)
                            smm(sp, 512, 768, 1, n_full + 2, 256)
                            pt = ptp.tile([128, 768], BF16, tag="pt",
                                          name="pt")
                            nc.scalar.activation(pt[:, 0:256], sp[:, 0:256],
                                                 EXP, scale=0.125)
                            nc.scalar.activation(pt[:, 512:768],
                                                 sp[:, 512:768],
                                                 EXP, scale=0.125)
                            nc.vector.tensor_mul(pt[:, 0:256], pt[:, 0:256],
                                                 mask_sb[:, 2048:2304])
                            nc.vector.tensor_mul(pt[:, 512:768],
                                                 pt[:, 512:768],
                                                 mask_sb[:, 2048:2304])
                        pts[idx] = pt

                    def emit_PV(idx):
                        pt = pts.pop(idx)
                        if idx < n_full + 1:
                            pvmm(0, idx, pt[:, 0:512], 0, False)
                            pvmm(1, idx, pt[:, 512:1024], 0, False)
                        elif idx == n_full + 1:
                            pvmm(0, n_full + 1, pt[:, 0:384], 128, False)
                            pvmm(1, n_full + 1, pt[:, 512:896], 128, False)
                            pvmm(0, n_full + 3, pt[:, 384:512], 384, False)
                            pvmm(1, n_full + 3, pt[:, 896:1024], 384, False)
                        else:                         # DC last: stop
                            pvmm(0, n_full + 2, pt[:, 0:256], 256, True)
                            pvmm(1, n_full + 2, pt[:, 512:768], 256, True)

                    for idx in range(nslots):
                        emit_S(idx)
                        if idx >= 1:
                            emit_PV(idx - 1)
                        filler()
                    emit_PV(nslots - 1)

                    # normalize: ctx = pv[0:64] * (1 / pv[64]); off the PE
                    # queue. The two DVE copies release pv's PSUM early; the
                    # recip/broadcast/mul chain then runs off SBUF. Custom
                    # DVE ops and partition_broadcast need base partition 0,
                    # so the l row is copied 64 -> 0 first.
                    for hl in range(2):
                        lrow = rawp.tile([1, 512], F32, tag="lrow",
                                         name="lrow")
                        nc.vector.tensor_copy(lrow[0:1, :],
                                              pv[hl][64:65, :])
                        raw = rawp.tile([64, 512], F32, tag="raw",
                                        name="raw")
                        nc.vector.tensor_copy(raw[:], pv[hl][0:64, :])
                        rf = rrowp.tile([1, 512], F32, tag="rf", name="rf")
                        nc.vector.reciprocal_approx_fast(
                            rf[0:1, :], lrow[0:1, :])
                        bcast = bcp.tile([64, 512], F32, tag="bc",
                                         name="bcast")
                        nc.gpsimd.partition_broadcast(bcast[0:64, :],
                                                      rf[0:1, :])
                        if hl == 0:
                            nc.vector.tensor_mul(
                                ctx_sb[hp][0:64, qbase:qbase + 512],
                                raw[:], bcast[:])
                        else:
                            tmp = tmpp.tile([64, 512], BF16, name="tmp")
                            nc.vector.tensor_mul(tmp[:], raw[:], bcast[:])
                            nc.sync.dma_start(
                                ctx_sb[hp][64:128, qbase:qbase + 512],
                                tmp[:])

                def project_out(tj):
                    tsl = slice(tj * 512, (tj + 1) * 512)
                    for oi in range(8):
                        osl = slice(oi * 128, (oi + 1) * 128)
                        ps_ = pps.tile([128, 512], F32, tag="pj",
                                       name="yacc")
                        for hp in range(HP):
                            nc.tensor.matmul(
                                ps_[:], wo_sb[hp][:, osl],
                                ctx_sb[hp][:, tsl],
                                start=(hp == 0), stop=(hp == HP - 1),
                                skip_group_check=True)
                        y_ = yp.tile([128, 512], F32)
                        nc.vector.tensor_scalar_add(
                            y_[:], ps_[:], bias_sb[:, oi:oi + 1])
                        nc.sync.dma_start(yT_d.ap()[osl, tsl], y_[:])

                # main schedule: V proj, then QK(hp0); per head pair,
                # attention with the next pair's QK groups as filler (one
                # group every 5th slot); the last pair takes the output
                # projection per query block instead.
                for fn in proj_group_fns(0):
                    fn()
                for hp in range(HP):
                    pending = proj_group_fns(hp + 1) if hp + 1 < HP else []
                    state = {"n": 0}

                    def filler():
                        state["n"] += 1
                        if pending and state["n"] % 5 == 2:
                            pending.pop(0)()
                    for qsb in range(NQSB):
                        attention_pair(hp, qsb, filler)
                        if hp == HP - 1:
                            project_out(qsb)
                    while pending:
                        pending.pop(0)()

            if iters == 1:
                emit()
            else:
                with tc.For_i(0, iters, 1):
                    emit()
    nc.compile()
    return nc


def make_masks():
    """Masks [128, MW2]: causal keep-bits for key row k = 128*j + k_local vs
    query q, laid out to match the packed psum tiles (A = j0|j0,
    B = j1,j3|j1,j3, C = j2)."""
    def blk(j):
        q = np.arange(QOFF[j], 512)[None, :]
        k = np.arange(128)[:, None]
        return (q >= 128 * j + k).astype(np.float32)
    b0, b1, b2, b3 = blk(0), blk(1), blk(2), blk(3)
    return np.concatenate([b0, b0, b1, b3, b1, b3, b2], axis=1)


def shard_inputs(x, w_qkv, w_out, b_out):
    """Full inputs -> list of 8 per-core input dicts (weights/x in bf16)."""
    import ml_dtypes
    bf16 = ml_dtypes.bfloat16
    x = np.asarray(x, dtype=np.float32)
    w_qkv = np.asarray(w_qkv, dtype=np.float32)
    w_out = np.asarray(w_out, dtype=np.float32)
    b_out = np.asarray(b_out, dtype=np.float32)
    masks = make_masks().astype(bf16)
    in_maps = []
    for c in range(N_CORES):
        b, hg = c // 2, c % 2
        h0 = hg * HPC
        csl = slice(h0 * D, (h0 + HPC) * D)
        im = {
            "xT": np.ascontiguousarray(x[b].T).astype(bf16),
            "wqT": np.ascontiguousarray(w_qkv[0 * C:1 * C][csl].T).astype(bf16),
            "wkT": np.ascontiguousarray(w_qkv[1 * C:2 * C][csl].T).astype(bf16),
            "wvT": np.ascontiguousarray(w_qkv[2 * C:3 * C][csl].T).astype(bf16),
            "woT": np.ascontiguousarray(w_out[:, csl].T).astype(bf16),
            "bias": (np.ascontiguousarray(b_out.reshape(8, 128).T)
                     if hg == 0 else np.zeros((128, 8), np.float32)),
            "masks": masks,
        }
        in_maps.append(im)
    return in_maps


def gather_outputs(results):
    """8 per-core {'yT': [C,T]} -> full [B,T,C]."""
    y = np.empty((B, T, C), np.float32)
    for b in range(B):
        acc = results[2 * b]["yT"] + results[2 * b + 1]["yT"]
        y[b] = acc.T
    return y


def kernel(**inputs):
    from concourse.bass_utils import run_bass_kernel_spmd
    if "nc" not in _CACHE:
        _CACHE["nc"] = build_nc()
    nc = _CACHE["nc"]
    in_maps = shard_inputs(inputs["x"], inputs["w_qkv"],
                           inputs["w_out"], inputs["b_out"])
    res = run_bass_kernel_spmd(nc, in_maps, list(range(N_CORES)))
    return gather_outputs(res.results)
